# revision 47
# baseline (speedup 1.0000x reference)
"""EvoAttentionCausal Trainium2 kernel (8-core SPMD).

Computes, per (b, h):
    v_swiglu = silu(V) * V
    c        = cumsum(v_swiglu, axis=S)
    c_n      = c * rsqrt(mean(c^2, D) + 1e-5)
    r        = ||Q||_D + ||K||_D + 1          (L2 eps 1e-8 is negligible)
    mstate   = c_n / r
    out0     = V + silu(mstate) * V
    out      = out0 * rsqrt(mean(out0^2, D) + 1e-5)

Sharding: B*H = 64 (b,h) units, 8 per NeuronCore, fully independent.

Layout: pair-packed sequence on partitions: s = 256*t + 2*p + e
(t = 16 meta-tiles, p = 128 partitions, e in {0,1}), so each partition's
HBM run is 2 rows = 512 B -> full-rate DMA descriptors.

Cumsum over S via tensor-engine matmuls (contract over partitions):
    c0[p,t] = carry[t] + L@w0 + Lstrict@w1      (e=0 positions)
    c1[p,t] = carry[t] + L@w0 + L@w1            (e=1 positions)
    colsums[t] = E_t^T@w0 + E_t^T@w1 ; carries = exclusive-prefix (Lx16)
    carry broadcast via esel[t]^T @ carries (PSUM accumulate)
Per-(s) scalars applied with stride-0 broadcast APs in tensor_tensor.
All rsqrt/sqrt on DVE (Newton + bit trick) so ACT stays on the
silu_and_others table set (Silu/Square/Copy/Identity) -> one table load.
"""

import sys

sys.path.insert(0, "/opt/trn_rl_repo")

import numpy as np

import concourse.bass as bass
import concourse.bacc as bacc
import concourse.mybir as mybir
import concourse.tile as tile
from concourse.bass_utils import run_bass_kernel_spmd

F32 = mybir.dt.float32
U32 = mybir.dt.uint32
AF = mybir.ActivationFunctionType
OP = mybir.AluOpType
AX = mybir.AxisListType

B, H, S, D = 4, 16, 4096, 64
NCORES = 8
BH = B * H              # 64
BHC = BH // NCORES      # 8 bh units per core
P = 128                 # partitions
TT = S // (2 * P)       # 16 meta-tiles (256 seq positions each)
NSC = 2 * TT            # 32 per-partition scalars per bh
GRP = 2                 # bh per row-math group
NG = BHC // GRP
RMS_EPS = 1e-5
CH = 8                  # meta-tiles per batched L-matmul (N=512)

_MAGIC = 0x5F3759DF


def _newton_rsqrt(nc, pool, x, magic, shift1, n, tag):
    """rsqrt(x) elementwise on a [128, n] f32 SBUF tile via bit trick +
    2 Newton iterations. Returns a fresh tile."""
    y = pool.tile([P, n], F32, tag=f"nwt_y_{tag}")
    t1 = pool.tile([P, n], F32, tag=f"nwt_t_{tag}")
    nc.vector.tensor_tensor(
        t1.bitcast(U32), x.bitcast(U32), shift1[:, 0:n], op=OP.logical_shift_right
    )
    nc.vector.tensor_tensor(
        y.bitcast(U32), magic[:, 0:n], t1.bitcast(U32), op=OP.subtract
    )
    # two Newton iterations, fused via scalar_tensor_tensor; the second
    # iteration cancels the sign flip of the first:
    #   y' = (0.5*x*y^2 - 1.5) * y  ==  -y_newton
    for _ in range(2):
        nc.vector.tensor_tensor(t1, y, y, op=OP.mult)
        nc.vector.scalar_tensor_tensor(t1, t1, 0.5, x, op0=OP.mult, op1=OP.mult)
        nc.vector.scalar_tensor_tensor(y, t1, 1.5, y, op0=OP.subtract, op1=OP.mult)
    return y


def build_consts():
    """Host-side constant matrices shipped as extra kernel inputs."""
    ut = np.triu(np.ones((P, P), np.float32))        # (L incl diag).T
    uts = np.triu(np.ones((P, P), np.float32), 1)    # (L strict).T
    # sliding-window col selector: E_t = slide[:, TT-1-t : 2*TT-1-t]
    slide = np.zeros((P, 2 * TT - 1), np.float32)
    slide[:, TT - 1] = 1.0
    sxt = np.triu(np.ones((TT, TT), np.float32), 1)  # exclusive prefix lhsT
    # esel[t]: [TT, 128] with row t all ones
    esel = np.zeros((TT, TT, P), np.float32)
    for t in range(TT):
        esel[t, t, :] = 1.0
    esel = esel.transpose(1, 0, 2).reshape(TT, TT * P)
    magic = np.full((P, 2 * GRP * NSC), _MAGIC, np.uint32)
    shift1 = np.full((P, 2 * GRP * NSC), 1, np.uint32)
    return {
        "c_ut": ut, "c_uts": uts, "c_slide": slide, "c_sxt": sxt,
        "c_esel": esel, "c_magic": magic, "c_shift1": shift1,
    }


def _pp(dram_bh):
    """Pair-packed view of one bh [S, D] slice: s = 256 t + 2 p + e."""
    return dram_bh.rearrange("(t p e) d -> p t e d", p=P, e=2)


def build_nc():
    nc = bacc.Bacc(
        "TRN2", target_bir_lowering=False, debug=False, num_devices=1
    )
    qd = nc.declare_dram_parameter("Q", [BHC, S, D], F32, isOutput=False)
    kd = nc.declare_dram_parameter("K", [BHC, S, D], F32, isOutput=False)
    vd = nc.declare_dram_parameter("V", [BHC, S, D], F32, isOutput=False)
    utd = nc.declare_dram_parameter("c_ut", [P, P], F32, isOutput=False)
    utsd = nc.declare_dram_parameter("c_uts", [P, P], F32, isOutput=False)
    slided = nc.declare_dram_parameter("c_slide", [P, 2 * TT - 1], F32, isOutput=False)
    sxtd = nc.declare_dram_parameter("c_sxt", [TT, TT], F32, isOutput=False)
    eseld = nc.declare_dram_parameter("c_esel", [TT, TT * P], F32, isOutput=False)
    magicd = nc.declare_dram_parameter("c_magic", [P, 2 * GRP * NSC], U32, isOutput=False)
    shiftd = nc.declare_dram_parameter("c_shift1", [P, 2 * GRP * NSC], U32, isOutput=False)
    outd = nc.declare_dram_parameter("out", [BHC, S, D], F32, isOutput=True)

    with tile.TileContext(nc) as tc:
        with (
            tc.tile_pool(name="consts", bufs=1) as consts,
            tc.tile_pool(name="vpool", bufs=3) as vpool,
            tc.tile_pool(name="cpool", bufs=2) as cpool,
            tc.tile_pool(name="opool", bufs=2) as opool,
            tc.tile_pool(name="scr", bufs=2) as scr,
            tc.tile_pool(name="small", bufs=2) as small,
            tc.tile_pool(name="rowm", bufs=2) as rowm,
            tc.tile_pool(name="pc0", bufs=3, space="PSUM") as ppc0,
            tc.tile_pool(name="pc1", bufs=3, space="PSUM") as ppc1,
            tc.tile_pool(name="pcs", bufs=1, space="PSUM") as ppcs,
            tc.tile_pool(name="pcar", bufs=1, space="PSUM") as ppcar,
        ):
            ut = consts.tile([P, P], F32)
            nc.sync.dma_start(out=ut, in_=utd[:, :])
            uts = consts.tile([P, P], F32)
            nc.sync.dma_start(out=uts, in_=utsd[:, :])
            slide = consts.tile([P, 2 * TT - 1], F32)
            nc.sync.dma_start(out=slide, in_=slided[:, :])
            sxt = consts.tile([TT, TT], F32)
            nc.sync.dma_start(out=sxt, in_=sxtd[:, :])
            esel = consts.tile([TT, TT * P], F32)
            nc.sync.dma_start(out=esel, in_=eseld[:, :])
            magic = consts.tile([P, 2 * GRP * NSC], U32)
            nc.sync.dma_start(out=magic, in_=magicd[:, :])
            shift1 = consts.tile([P, 2 * GRP * NSC], U32)
            nc.sync.dma_start(out=shift1, in_=shiftd[:, :])

            for g in range(NG):
                bhs = [g * GRP + i for i in range(GRP)]
                n = GRP * NSC
                ccg = rowm.tile([P, n], F32, tag="ccg")
                qkg = rowm.tile([P, 2 * n], F32, tag="qkg")
                qqg = qkg[:, 0:n]
                kkg = qkg[:, n:2 * n]

                vts, cts = {}, {}
                for i, bh in enumerate(bhs):
                    vt = vpool.tile([P, TT, 2, D], F32, tag="v")
                    nc.sync.dma_start(out=vt, in_=_pp(vd[bh]))
                    vts[bh] = vt
                    sv = scr.tile([P, TT, 2, D], F32, tag="sv")
                    wt = scr.tile([P, TT, 2, D], F32, tag="w")
                    for ci in range(2):
                        tsl = slice(ci * CH, (ci + 1) * CH)
                        nc.scalar.activation(sv[:, tsl], vt[:, tsl], AF.Silu)
                        nc.gpsimd.tensor_tensor(
                            wt[:, tsl], sv[:, tsl], vt[:, tsl], op=OP.mult)

                    # Q/K norms^2 (per (t, e) row)
                    for dram, dst, sqeng in (
                            (qd, qqg, "act" if i == 0 else "pool"),
                            (kd, kkg, "pool" if i == 0 else "act")):
                        xt = scr.tile([P, TT, 2, D], F32, tag="qk")
                        nc.sync.dma_start(out=xt, in_=_pp(dram[bh]))
                        sq = scr.tile([P, TT, 2, D], F32, tag="sq")
                        if sqeng == "act":
                            nc.scalar.square(sq, xt)
                        else:
                            nc.gpsimd.tensor_tensor(sq, xt, xt, op=OP.mult)
                        nc.vector.tensor_reduce(
                            dst[:, i * NSC:(i + 1) * NSC],
                            sq, axis=AX.X, op=OP.add,
                        )

                    w1 = wt[:, :, 1, :]
                    # u = w0 + w1 (pair sums)
                    ux = scr.tile([P, TT, D], F32, tag="u")
                    nc.gpsimd.tensor_tensor(
                        ux, wt[:, :, 0, :], w1, op=OP.add)

                    # colsums[t] = sum_p u[p, t]
                    pcs = ppcs.tile([TT, D], F32)
                    for t in range(TT):
                        win = slide[:, TT - 1 - t:2 * TT - 1 - t]
                        nc.tensor.matmul(pcs, win, ux[:, t, :],
                                         start=(t == 0), stop=(t == TT - 1))
                    cs_s = small.tile([TT, D], F32, tag="cs")
                    nc.scalar.copy(cs_s, pcs)
                    pcar = ppcar.tile([TT, D], F32)
                    nc.tensor.matmul(pcar, sxt, cs_s, start=True, stop=True)
                    car_s = small.tile([TT, D], F32, tag="car")
                    nc.scalar.copy(car_s, pcar)

                    # c1 = carry + L@u per chunk; c0 = c1 - w1 elementwise
                    ct = cpool.tile([P, TT, 2, D], F32, tag="c")
                    cts[bh] = ct
                    for ci in range(TT // CH):
                        tsl = slice(ci * CH, (ci + 1) * CH)
                        pc1 = ppc1.tile([P, CH, D], F32)
                        nc.tensor.matmul(pc1, ut, ux[:, tsl, :],
                                         start=True, stop=False)
                        for tt_ in range(CH):
                            t = ci * CH + tt_
                            es = esel[:, t * P:(t + 1) * P]
                            nc.tensor.matmul(pc1[:, tt_, :], es, car_s,
                                             start=False, stop=(tt_ == CH - 1))
                        nc.scalar.copy(ct[:, tsl, 1, :], pc1)
                        nc.gpsimd.tensor_tensor(
                            ct[:, tsl, 0, :], ct[:, tsl, 1, :], w1[:, tsl, :],
                            op=OP.subtract)

                    # cc = sum_d c^2 per (t, e)
                    sqc = scr.tile([P, TT, 2, D], F32, tag="sq")
                    for ci in range(2):
                        tsl = slice(ci * CH, (ci + 1) * CH)
                        nc.scalar.square(sqc[:, tsl], ct[:, tsl])
                        nc.vector.tensor_reduce(
                            ccg[:, i * NSC + ci * CH * 2:
                                i * NSC + (ci + 1) * CH * 2],
                            sqc[:, tsl], axis=AX.X, op=OP.add,
                        )

                # --- group row math: s1 = rsqrt((cc/64+eps) * r^2) ---
                rqk = _newton_rsqrt(nc, rowm, qkg, magic, shift1, 2 * n, "qk")
                nc.vector.tensor_tensor(rqk, rqk, qkg, op=OP.mult)  # sqrt
                r = rowm.tile([P, n], F32, tag="r")
                nc.vector.scalar_tensor_tensor(
                    r, rqk[:, 0:n], 1.0, rqk[:, n:2 * n], op0=OP.add, op1=OP.add
                )
                ccp = rowm.tile([P, n], F32, tag="ccp")
                nc.vector.tensor_scalar(
                    ccp, ccg, 1.0 / D, RMS_EPS, op0=OP.mult, op1=OP.add
                )
                nc.vector.tensor_tensor(ccp, ccp, r, op=OP.mult)
                nc.vector.tensor_tensor(ccp, ccp, r, op=OP.mult)
                s1 = _newton_rsqrt(nc, rowm, ccp, magic, shift1, n, "s1")

                oog = rowm.tile([P, n], F32, tag="oog")
                o0s = {}
                for i, bh in enumerate(bhs):
                    ct, vt = cts[bh], vts[bh]
                    s1b = s1[:, i * NSC:(i + 1) * NSC].rearrange(
                        "p (t e) -> p t e", e=2)[:, :, :, None].broadcast_to(
                        [P, TT, 2, D])
                    ms = scr.tile([P, TT, 2, D], F32, tag="ms")
                    gt = scr.tile([P, TT, 2, D], F32, tag="g")
                    o0 = opool.tile([P, TT, 2, D], F32, tag="o0")
                    for ci in range(2):
                        tsl = slice(ci * CH, (ci + 1) * CH)
                        nc.gpsimd.tensor_tensor(
                            ms[:, tsl], ct[:, tsl], s1b[:, tsl], op=OP.mult)
                        nc.scalar.activation(gt[:, tsl], ms[:, tsl], AF.Silu)
                        # gate = g+1 at DVE 2x rate, then mult on GPSIMD
                        nc.vector.tensor_scalar(
                            gt[:, tsl], gt[:, tsl], 1.0, None, op0=OP.add)
                        nc.gpsimd.tensor_tensor(
                            o0[:, tsl], gt[:, tsl], vt[:, tsl], op=OP.mult)
                    o0s[bh] = o0
                    sqo = scr.tile([P, TT, 2, D], F32, tag="sq")
                    for ci in range(2):
                        tsl = slice(ci * CH, (ci + 1) * CH)
                        if i == 0:
                            nc.scalar.square(sqo[:, tsl], o0[:, tsl])
                        else:
                            nc.gpsimd.tensor_tensor(
                                sqo[:, tsl], o0[:, tsl], o0[:, tsl],
                                op=OP.mult)
                        nc.vector.tensor_reduce(
                            oog[:, i * NSC + ci * CH * 2:
                                i * NSC + (ci + 1) * CH * 2],
                            sqo[:, tsl], axis=AX.X, op=OP.add,
                        )

                oop = rowm.tile([P, n], F32, tag="ccp")
                nc.vector.tensor_scalar(
                    oop, oog, 1.0 / D, RMS_EPS, op0=OP.mult, op1=OP.add
                )
                s2 = _newton_rsqrt(nc, rowm, oop, magic, shift1, n, "s2")

                for i, bh in enumerate(bhs):
                    o0 = o0s[bh]
                    s2b = s2[:, i * NSC:(i + 1) * NSC].rearrange(
                        "p (t e) -> p t e", e=2)[:, :, :, None].broadcast_to(
                        [P, TT, 2, D])
                    ot = opool.tile([P, TT, 2, D], F32, tag="ot")
                    op_ = _pp(outd[bh])
                    for ci in range(2):
                        tsl = slice(ci * CH, (ci + 1) * CH)
                        nc.gpsimd.tensor_tensor(
                            ot[:, tsl], o0[:, tsl], s2b[:, tsl], op=OP.mult)
                        nc.sync.dma_start(out=op_[:, tsl], in_=ot[:, tsl])
    nc.finalize()
    return nc


_NC_CACHE = None


def kernel(Q, K, V):
    global _NC_CACHE
    if _NC_CACHE is None:
        _NC_CACHE = build_nc()
    nc = _NC_CACHE
    consts = build_consts()
    Qs = np.ascontiguousarray(np.asarray(Q, np.float32).reshape(BH, S, D))
    Ks = np.ascontiguousarray(np.asarray(K, np.float32).reshape(BH, S, D))
    Vs = np.ascontiguousarray(np.asarray(V, np.float32).reshape(BH, S, D))
    in_maps = []
    for c in range(NCORES):
        sl = slice(c * BHC, (c + 1) * BHC)
        in_maps.append({"Q": Qs[sl], "K": Ks[sl], "V": Vs[sl], **consts})
    res = run_bass_kernel_spmd(nc, in_maps, list(range(NCORES)))
    out = np.concatenate([res.results[c]["out"] for c in range(NCORES)], axis=0)
    return out.reshape(B, H, S, D)
